# revision 65
# baseline (speedup 1.0000x reference)
"""CoordAtt Trainium2 Bass kernel.

Reference computation (per batch n, c=256, h=w=64, mip=8):
    xs   = x + residual                      (bilinear resize of residual at
                                              identical shape is the identity)
    y    = concat(mean_w(xs), mean_h(xs))    -> [c, h+w]
    y    = hswish(BN(w1 @ y + b1))           -> [mip, h+w]
    a_h  = sigmoid(w2 @ y[:, :h] + b2)       -> [c, h]
    a_w  = sigmoid(w3 @ y[:, h:] + b3)       -> [c, w]
    out  = 2*xs*a_h*a_w + 2*residual*(1 - a_h*a_w)
         = 2*(a_h*a_w*x + residual)          (algebraically identical)

Kernel strategy (8 cores, data-parallel over batch n: 2 batches/core).
The kernel is HBM-bound: 16.8 MB in + 8.4 MB out per core at 360 GB/s
(~70 us) sets the floor, so everything is organized to keep the DMA
engines saturated end to end:
  * conv-before-pool: pooling and the 1x1 conv are both linear, so compute
    y_conv = w1^T @ x + w1^T @ res on the TensorEngine (PSUM accumulation
    gives the x+res add for free), then pool the tiny (mip, h*w) result.
    Matmuls run in float32r: fp32 data at the full PE rate (no bf16 casts,
    which keeps the Activation engine out of the streaming path entirely).
  * b2/b3 biases ride a ones-row appended to the MLP activations so the
    attention matmuls add them in PSUM; hswish's /6 is folded into w2/w3;
    the final x2 scaling is folded into scalar_tensor_tensor immediates.
  * the BN + hswish MLP chain runs entirely on DVE (scalar_tensor_tensor +
    dual-op tensor_scalar), avoiding cross-engine hops on the a_w critical
    path; sigmoids run on the idle Activation engine.
  * final elementwise is 3 fused stt passes per tile, split per-unit across
    DVE and GpSimd (GpSimd's TensorScalarPtr runs at 0.6 efficiency vs 0.42
    for plain TensorTensor, so every pass is an stt).
  * emission order pipelines batches: batch 0's finals and stores are queued
    on every engine BEFORE batch 1's phase-1 work, so output DMAs flow as
    soon as the input stream ends; stores are emitted in predicted
    readiness order to keep the (serializing) DMA device gap-free.
"""

import numpy as np

import concourse.bacc as bacc
import concourse.mybir as mybir
from concourse.tile import TileContext
from concourse.bass_utils import run_bass_kernel_spmd
from concourse.masks import make_identity

F32 = mybir.dt.float32
F32R = mybir.dt.float32r
BF16 = mybir.dt.bfloat16
Alu = mybir.AluOpType
Act = mybir.ActivationFunctionType
AX = mybir.AxisListType

N_CORES = 8
N, C, H, W = 16, 256, 64, 64
NLOC = N // N_CORES           # batches per core
MIP = 8
EPS = 1e-5
HW = H * W                    # 4096 free columns per (batch, c-chunk)
NCHUNK = C // 128             # c-chunk count (2)
NHALF = 2                     # h-half split of each chunk tile
HCOL = HW // NHALF            # 2048 columns per half tile
HALFH = H // NHALF            # h rows per half tile (32)
NSEG = 4                      # conv psum segments per batch: 4 x 1024 cols
SEGH = H // NSEG              # h rows per segment (16)
SEGCOL = SEGH * W             # columns per segment (1024)

# unit = (k, s) c-chunk/h-quarter pair; GpSimd (Pool) takes the k=0 units,
# DVE (which also owns the reduces) the k=1 units, for both the a_h multiply
# and the final passes
GP_UNITS = frozenset({(0, 0), (0, 1), (0, 2), (0, 3)})
# store order within a batch, matched to when each unit's finals land
STORE_ORDER = ((1, 0), (0, 0), (1, 1), (0, 1), (1, 2), (0, 2), (1, 3), (0, 3))


def build_module():
    nc = bacc.Bacc("TRN2", target_bir_lowering=False)

    x_d = nc.dram_tensor("x", (NLOC, C, H, W), F32, kind="ExternalInput")
    r_d = nc.dram_tensor("residual", (NLOC, C, H, W), F32, kind="ExternalInput")
    w1_d = nc.dram_tensor("w1", (MIP, C), F32, kind="ExternalInput")
    b1_d = nc.dram_tensor("b1", (MIP,), F32, kind="ExternalInput")
    gamma_d = nc.dram_tensor("bn_gamma", (MIP,), F32, kind="ExternalInput")
    beta_d = nc.dram_tensor("bn_beta", (MIP,), F32, kind="ExternalInput")
    mean_d = nc.dram_tensor("bn_mean", (MIP,), F32, kind="ExternalInput")
    var_d = nc.dram_tensor("bn_var", (MIP,), F32, kind="ExternalInput")
    w2_d = nc.dram_tensor("w2", (C, MIP), F32, kind="ExternalInput")
    b2_d = nc.dram_tensor("b2", (C,), F32, kind="ExternalInput")
    w3_d = nc.dram_tensor("w3", (C, MIP), F32, kind="ExternalInput")
    b3_d = nc.dram_tensor("b3", (C,), F32, kind="ExternalInput")
    out_d = nc.dram_tensor("out", (NLOC, C, H, W), F32, kind="ExternalOutput")

    with TileContext(nc) as tc:
        with (
            tc.tile_pool(name="big", bufs=1) as big,
            tc.tile_pool(name="small", bufs=1) as small,
            tc.tile_pool(name="work", bufs=2) as work,
            tc.tile_pool(name="psum_y", bufs=2, space="PSUM") as psum_y_pool,
            tc.tile_pool(name="psum_a", bufs=1, space="PSUM") as psum_a_pool,
        ):
            # ---- replicated constants ----
            # weights are loaded in their natural (contiguous) DRAM layout —
            # a transposed DMA costs one descriptor per element (~900 ns of
            # DMA-device time each); instead transpose on the PE against an
            # identity, off the streaming path entirely.
            ident = small.tile([128, 128], F32, tag="ident")
            make_identity(nc, ident[:])
            w1n = small.tile([MIP, C], F32, tag="w1n")
            nc.scalar.dma_start(w1n[:], w1_d[:, :])
            w2n = small.tile([128, 2 * MIP], F32, tag="w2n")
            nc.scalar.dma_start(w2n[:].rearrange("p (k m) -> p k m", k=2),
                    w2_d.rearrange("(k p) m -> p k m", p=128))
            w3n = small.tile([128, 2 * MIP], F32, tag="w3n")
            nc.scalar.dma_start(w3n[:].rearrange("p (k m) -> p k m", k=2),
                    w3_d.rearrange("(k p) m -> p k m", p=128))
            # w1 chunk-transposed: (c128, mip) per c-chunk, fp32 (f32r matmul)
            w1t = []
            w1tf = []
            trp = psum_a_pool.tile([128, 128], F32, tag="tr")
            for k in range(NCHUNK):
                t = small.tile([128, MIP], BF16, name=f"w1t{k}", tag=f"w1t{k}")
                tf = small.tile([128, MIP], F32, name=f"w1tf{k}", tag=f"w1tf{k}")
                nc.tensor.transpose(trp[:, :MIP], w1n[:, k * 128:(k + 1) * 128],
                                    ident[:MIP, :MIP])
                nc.scalar.copy(t[:], trp[:, :MIP])
                nc.scalar.copy(tf[:], trp[:, :MIP])
                w1t.append(t)
                w1tf.append(tf)
            # w2/w3 transposed with bias row: (mip+1, C); row MIP carries
            # b2/b3 so the attention matmul adds the bias in PSUM
            w2e = small.tile([MIP + 1, C], F32, tag="w2e")
            w3e = small.tile([MIP + 1, C], F32, tag="w3e")
            for wn, we in ((w2n, w2e), (w3n, w3e)):
                for k in range(NCHUNK):
                    nc.tensor.transpose(trp[:MIP, :], wn[:, k * MIP:(k + 1) * MIP],
                                        ident[:])
                    nc.scalar.copy(we[:MIP, k * 128:(k + 1) * 128], trp[:MIP, :128])
            nc.scalar.dma_start(w2e[MIP:MIP + 1, :], b2_d[:].unsqueeze(0))
            nc.scalar.dma_start(w3e[MIP:MIP + 1, :], b3_d[:].unsqueeze(0))
            # BN constants, (mip, 1) per-partition scalars
            bn_in = small.tile([MIP, 5], F32, tag="bn_in")
            for i, d in enumerate((var_d, gamma_d, beta_d, mean_d, b1_d)):
                nc.gpsimd.dma_start(bn_in[:, i:i + 1], d[:].unsqueeze(1))
            var_c = bn_in[:, 0:1]
            gamma_c = bn_in[:, 1:2]
            beta_c = bn_in[:, 2:3]
            mean_c = bn_in[:, 3:4]
            b1_c = bn_in[:, 4:5]

            # fold hswish's /6 into the attention weights (bias row excluded)
            nc.scalar.mul(w2e[:MIP, :], w2e[:MIP, :], 1.0 / 6.0)
            nc.scalar.mul(w3e[:MIP, :], w3e[:MIP, :], 1.0 / 6.0)

            bn_t = small.tile([MIP, 5], F32, tag="bn_t")
            sv = bn_t[:, 0:1]       # sqrt(var+eps)
            inv = bn_t[:, 1:2]      # gamma / sqrt(var+eps)
            scale_p = bn_t[:, 2:3]  # inv / W   (pool-sum -> mean fold)
            bias_p = bn_t[:, 3:4]   # (b1 - mean) * inv + beta
            eps_c = bn_t[:, 4:5]
            nc.vector.memset(eps_c, EPS)
            nc.scalar.activation(sv, var_c, Act.Sqrt, bias=eps_c, scale=1.0)
            nc.vector.reciprocal(inv, sv)
            nc.vector.tensor_tensor(inv, inv, gamma_c, Alu.mult)
            nc.vector.tensor_scalar_mul(scale_p, inv, 1.0 / W)
            nc.vector.tensor_tensor(bias_p, b1_c, mean_c, Alu.subtract)
            nc.vector.scalar_tensor_tensor(bias_p, bias_p, inv, beta_c, Alu.mult, Alu.add)

            # ---- all input loads upfront (SP queue) ----
            # quarter-granular tiles: each (chunk k, segment s) is its own
            # SBUF tensor, so DVE and GpSimd can split finals freely without
            # ever co-writing one tensor (a known device-hang hazard)
            xt = {}
            rt = {}
            for b in range(NLOC):
                for s in range(NSEG):
                    js = slice(s * SEGCOL, (s + 1) * SEGCOL)
                    for src_d, store, nm in ((r_d, rt, "r"), (x_d, xt, "x")):
                        for k in range(NCHUNK):
                            cs = slice(k * 128, (k + 1) * 128)
                            t = big.tile([128, SEGCOL], F32, name=f"{nm}_{b}_{k}_{s}",
                                         tag=f"{nm}{b}{k}{s}")
                            nc.sync.dma_start(t[:], src_d[b, cs].rearrange("c h w -> c (h w)")[:, js])
                            store[b, k, s] = t

            def unit_engine(k, s):
                return nc.gpsimd if (k, s) in GP_UNITS else nc.vector

            # bf16 casts of every quarter tile on the otherwise-idle
            # Activation engine — the conv's inputs.  (FP32R direct from DMA
            # is rejected by the BIR verifier: every producer of an
            # f32r-matmult input must itself round to f32r, which GpSimd
            # can't do.)  x/r stay fp32 for the finals.
            xbt = {}
            rbt = {}
            for b in range(NLOC):
                for s in range(NSEG - 1):
                    for store, bstore, nm in ((rt, rbt, "rb"), (xt, xbt, "xb")):
                        for k in range(NCHUNK):
                            t = big.tile([128, SEGCOL], BF16, name=f"{nm}_{b}_{k}_{s}",
                                         tag=f"{nm}{k}", bufs=8)
                            with tc.high_priority(offset=10_000_000):
                                nc.scalar.copy(t[:], store[b, k, s][:])
                            bstore[b, k, s] = t

            # ---- per batch: conv -> pools -> mlp -> attention -> finals ----
            for b in range(NLOC):
                # row MIP stays 1.0 (bias row for the attention matmuls);
                # memset the whole tile — a partition-8-start AP would
                # violate the BIR partition-alignment rule
                vfull = work.tile([MIP + 1, H + W], F32, name=f"v_{b}", tag="vfull")
                nc.vector.memset(vfull[:], 1.0)
                yh_sum = work.tile([MIP, H], F32, name=f"yh_{b}", tag="yh")
                ywp = work.tile([MIP, NSEG * W], F32, name=f"ywp_{b}", tag="ywp")
                ah = {}
                for k in range(NCHUNK):
                    ah[k] = work.tile([128, H], BF16, name=f"ah_{b}_{k}", tag=f"ah{k}")

                def emit_ah(j):
                    # a_h MLP for half j, on DVE (scalar_tensor_tensor is
                    # only supported by walrus on DVE, not GpSimd):
                    # ybn = BN(y); u = max(ybn+3, 0); v = min(u,6)*ybn.
                    # The sigmoid result is doubled on Act so the ph1/ph2
                    # passes are plain multiplies on either engine.
                    e = nc.vector
                    hs = slice(j * HALFH, (j + 1) * HALFH)
                    ybn = work.tile([MIP, HALFH], F32, name=f"ybnh_{b}_{j}", tag="ybnh", bufs=4)
                    u = work.tile([MIP, HALFH], F32, name=f"uh_{b}_{j}", tag="uh", bufs=4)
                    with tc.high_priority(offset=5_000_000):
                        e.scalar_tensor_tensor(
                            ybn[:], yh_sum[:, hs], scale_p,
                            bias_p.broadcast_to((MIP, HALFH)), Alu.mult, Alu.add)
                        e.tensor_scalar(u[:], ybn[:], 3.0, 0.0, Alu.add, Alu.max)
                        e.scalar_tensor_tensor(
                            vfull[:MIP, hs], u[:], 6.0, ybn[:], Alu.min, Alu.mult)
                    with tc.high_priority(offset=15_000_000):
                        for k in range(NCHUNK):
                            cs = slice(k * 128, (k + 1) * 128)
                            ahp = psum_a_pool.tile([128, HALFH], F32, name=f"ahp_{b}_{j}_{k}", tag="ahp")
                            nc.tensor.matmul(ahp[:], w2e[:, cs], vfull[:, hs], start=True, stop=True)
                            nc.scalar.activation(ah[k][:, hs], ahp[:], Act.Sigmoid)

                def emit_ph1(j):
                    # first final pass: x = (2x) * a_h, per quarter-unit.
                    # Almost all ph1 runs on Pool (idle while DVE owns the
                    # reduce -> a_w critical chain); DVE only takes (1,2) and
                    # (1,3) to balance the post-a_w tail.  Cross-engine ph1 ->
                    # ph2 handoffs are dependency-serialized (never written
                    # concurrently by both engines).
                    for s in (2 * j, 2 * j + 1):
                        srows = slice(s * SEGH, (s + 1) * SEGH)
                        for k in range(NCHUNK):
                            eng = nc.vector if (k, s) in ((1, 2), (1, 3)) else nc.gpsimd
                            xsrc = xt[b, k, s] if s == NSEG - 1 else xbt[b, k, s]
                            xs_ = xsrc.rearrange("p (h w) -> p h w", h=SEGH)
                            ahb = ah[k][:, srows].unsqueeze(2).broadcast_to((128, SEGH, W))
                            eng.tensor_tensor(xs_, xs_, ahb, Alu.mult)

                ypsum_h1 = {}
                for s in range(NSEG):
                    j = s // (NSEG // NHALF)
                    # conv (c -> mip) with implicit x+res via PSUM accumulation
                    ypsum = psum_y_pool.tile([MIP, SEGCOL], F32, name=f"yp_{b}_{s}", tag="yp")
                    if s == NSEG - 1:
                        srcs = [(w1tf[0], rt[b, 0, s]), (w1tf[1], rt[b, 1, s]),
                                (w1tf[0], xt[b, 0, s]), (w1tf[1], xt[b, 1, s])]
                    else:
                        srcs = [(w1t[0], rbt[b, 0, s]), (w1t[1], rbt[b, 1, s]),
                                (w1t[0], xbt[b, 0, s]), (w1t[1], xbt[b, 1, s])]
                    # column groups interleaved: after the segment's last
                    # tile lands only its 2 closing matmuls remain, not 5
                    with tc.high_priority(offset=10_000_000):
                        for i, (wt, src) in enumerate(srcs):
                            for jj in range(0, SEGCOL, 512):
                                nc.tensor.matmul(
                                    ypsum[:, jj:jj + 512],
                                    wt[:],
                                    src[:, jj:jj + 512],
                                    start=(i == 0),
                                    stop=(i == len(srcs) - 1),
                                )
                    # directional pool sums for this segment (DVE); the w-pool
                    # partial first — it gates a_w, the batch critical path.
                    # For the last half the h-pools (a_h path) are deferred
                    # until after the a_w chain is queued. High priority:
                    # these tiny ops gate the whole attention chain and must
                    # preempt queued full-size passes on DVE.
                    with tc.high_priority(offset=10_000_000):
                        nc.vector.reduce_sum(
                            ywp[:, s * W:(s + 1) * W],
                            ypsum.rearrange("m (h w) -> m w h", h=SEGH),
                            axis=AX.X,
                        )
                    with tc.high_priority(offset=5_000_000):
                        if j == 0:
                            nc.vector.reduce_sum(
                                yh_sum[:, s * SEGH:(s + 1) * SEGH],
                                ypsum.rearrange("m (h w) -> m h w", h=SEGH),
                                axis=AX.X,
                            )
                    if j == 1:
                        ypsum_h1[s] = ypsum
                    if s == 1:
                        emit_ah(0)
                        emit_ph1(0)

                # a_w path (critical for the batch tail): pools -> MLP -> conv
                yw_sum = work.tile([MIP, W], F32, name=f"yw_{b}", tag="yw")
                ybnw = work.tile([MIP, W], F32, name=f"ybnw_{b}", tag="ybnw")
                uw = work.tile([MIP, W], F32, name=f"uw_{b}", tag="uw")
                aw = {}
                with tc.high_priority(offset=10_000_000):
                    nc.vector.tensor_tensor(yw_sum[:], ywp[:, 0:W], ywp[:, W:2 * W], Alu.add)
                    nc.vector.tensor_tensor(ywp[:, 2 * W:3 * W], ywp[:, 2 * W:3 * W],
                                            ywp[:, 3 * W:4 * W], Alu.add)
                    nc.vector.tensor_tensor(yw_sum[:], yw_sum[:], ywp[:, 2 * W:3 * W], Alu.add)
                    nc.vector.scalar_tensor_tensor(
                        ybnw[:], yw_sum[:], scale_p,
                        bias_p.broadcast_to((MIP, W)), Alu.mult, Alu.add)
                    nc.vector.tensor_scalar(uw[:], ybnw[:], 3.0, 0.0, Alu.add, Alu.max)
                    nc.vector.scalar_tensor_tensor(
                        vfull[:MIP, H:], uw[:], 6.0, ybnw[:], Alu.min, Alu.mult)
                    with tc.high_priority(offset=15_000_000):
                        for k in range(NCHUNK):
                            cs = slice(k * 128, (k + 1) * 128)
                            awt = work.tile([128, W], BF16, name=f"aw_{b}_{k}", tag=f"aw{k}")
                            awp = psum_a_pool.tile([128, W], F32, name=f"awp_{b}_{k}", tag="awp")
                            nc.tensor.matmul(awp[:], w3e[:, cs], vfull[:, H:], start=True, stop=True)
                            nc.scalar.activation(awt[:], awp[:], Act.Sigmoid)
                            # x2 of the output identity folded into a_w once
                            nc.scalar.mul(awt[:], awt[:], 2.0)
                            aw[k] = awt

                # a_h h-pools for half 1 (band B: after the a_w path)
                with tc.high_priority(offset=5_000_000):
                    for s in (2, 3):
                        nc.vector.reduce_sum(
                            yh_sum[:, s * SEGH:(s + 1) * SEGH],
                            ypsum_h1[s].rearrange("m (h w) -> m h w", h=SEGH),
                            axis=AX.X,
                        )
                emit_ah(1)
                emit_ph1(1)

                # final passes + stores, quarter-granular so the store stream
                # starts as early as possible: x *= a_w ; r = 2r + x ; store r
                # the r-pass (which gates the store) runs in a priority band
                # above the x-passes: when a unit's x-pass finishes, its
                # r-pass preempts the next unit's x-pass in the ready heap,
                # so stores complete one-per-unit instead of after all x's
                for k, s in STORE_ORDER:
                    eng = unit_engine(k, s)
                    cs = slice(k * 128, (k + 1) * 128)
                    od = out_d[b, cs].rearrange("c h w -> c (h w)")
                    xsrc = xt[b, k, s] if s == NSEG - 1 else xbt[b, k, s]
                    xs_ = xsrc.rearrange("p (h w) -> p h w", h=SEGH)
                    rs_ = rt[b, k, s].rearrange("p (h w) -> p h w", h=SEGH)
                    awb = aw[k].unsqueeze(1).broadcast_to((128, SEGH, W))
                    eng.tensor_tensor(xs_, xs_, awb, Alu.mult)
                    with tc.high_priority(offset=2_000_000):
                        with tc.high_priority(offset=-1_000_000):
                        nc.vector.scalar_tensor_tensor(rs_, rs_, 2.0, xs_, Alu.mult, Alu.add)
                    nc.sync.dma_start(
                        od[:, s * SEGCOL:(s + 1) * SEGCOL], rt[b, k, s][:])

    nc.compile()
    return nc


_NC_CACHE = None


def _get_module():
    global _NC_CACHE
    if _NC_CACHE is None:
        _NC_CACHE = build_module()
    return _NC_CACHE


def make_in_maps(inputs):
    reps = {k: np.ascontiguousarray(v) for k, v in inputs.items()
            if k not in ("x", "residual")}
    in_maps = []
    for core in range(N_CORES):
        bs = slice(core * NLOC, (core + 1) * NLOC)
        m = {"x": np.ascontiguousarray(inputs["x"][bs]),
             "residual": np.ascontiguousarray(inputs["residual"][bs])}
        m.update(reps)
        in_maps.append(m)
    return in_maps


def run_spmd(nc, in_maps):
    res = run_bass_kernel_spmd(nc, in_maps, core_ids=list(range(N_CORES)))
    return np.concatenate([res.results[c]["out"] for c in range(N_CORES)], axis=0)


def kernel(**inputs):
    inputs = {k: np.asarray(v) for k, v in inputs.items()}
    nc = _get_module()
    return run_spmd(nc, make_in_maps(inputs))
